# revision 7
# baseline (speedup 1.0000x reference)
"""Distributed Trainium2 kernel for nn_AttentionHead (B=8, N=2048, H=E=1024).

Single attention head with an UPPER-triangular mask (reference masks i > j,
i.e. position i attends to j >= i), softmax over j, applied per batch:

    K = X Wk; Q = X Wq; V = X Wv
    S = Q K^T / sqrt(E);  S[i, j] = -inf for i > j
    O = softmax_j(S) V

Sharding: pure data parallel -- batch b (8) maps 1:1 onto the 8 NeuronCores.
Weights replicated; no collectives.

Per-core algorithm (v3; all matmuls bf16 with fp32 PSUM accumulation):
  - X^T is produced by the DMA xbar (one DMA_TRANSPOSE per [128, 1024]
    chunk with a 3D output AP, all on the Sync HWDGE queue -- concurrent
    xbar ops on both HWDGE queues corrupt data). f32->bf16 casts happen
    inside the SWDGE load DMAs. The PE does nothing but matmuls.
  - Projections Q^T[e,i], K^T[e,j], V[j,e] use the weights in NATURAL
    layout as lhsT (no weight transposes at all):
        qt = matmul(lhsT=Wq[h, e-slice], rhs=X^T[h, i])   etc.
  - The attention phase is computed TRANSPOSED: S^T tiles [128 j, 512 i]
    from matmul(lhsT=K^T, rhs=Q^T); exp writes P^T which is exactly the
    lhsT the PV matmul needs (no per-tile PE transposes of P). Row sums
    fall out of a [128,1] ones-matmul sharing the PV stationary weights.
  - Triangular structure is skipped at 128-col granularity on both the
    S^T and PV sides; the diagonal 128x128 gets an additive -1e30 mask.
"""

import numpy as np

try:
    import concourse.bass as bass
except ImportError:  # fresh grading dir: concourse comes from the site repo
    import sys

    for p in ("/opt/trn_rl_repo", "/root/.axon_site/_ro/trn_rl_repo"):
        if p not in sys.path:
            sys.path.append(p)
    import concourse.bass as bass

import concourse.mybir as mybir
import concourse.tile as tile
from concourse import bacc, bass_utils

B, N, H, E = 8, 2048, 1024, 1024
P = 128
HT = H // P  # 8 h-tiles
ET = E // P  # 8 e-tiles
NT = N // P  # 16 row tiles
IB = 512  # i-block width in the attention phase
NIB = N // IB  # 4
F32 = mybir.dt.float32
BF16 = mybir.dt.bfloat16
SCALE = 1.0 / float(np.sqrt(E))
NEG = -1.0e30


def build_graph():
    nc = bacc.Bacc("TRN2", target_bir_lowering=False, debug=False,
                   enable_asserts=False)
    x = nc.dram_tensor("input", [N, H], F32, kind="ExternalInput").ap()
    wk = nc.dram_tensor("k", [H, E], F32, kind="ExternalInput").ap()
    wq = nc.dram_tensor("q", [H, E], F32, kind="ExternalInput").ap()
    wv = nc.dram_tensor("v", [H, E], F32, kind="ExternalInput").ap()
    out = nc.dram_tensor("out", [N, E], F32, kind="ExternalOutput").ap()

    with tile.TileContext(nc) as tc:
        with (
            tc.tile_pool(name="const", bufs=1) as constp,
            tc.tile_pool(name="persist", bufs=1) as persist,
            tc.tile_pool(name="psMM", bufs=3, space="PSUM") as psMM,
            tc.tile_pool(name="psO", bufs=4, space="PSUM") as psO,
            tc.tile_pool(name="psR", bufs=1, space="PSUM") as psR,
        ):
            maskt = constp.tile([P, P], F32)
            ones = constp.tile([P, 1], BF16)

            xt = persist.tile([P, HT, N], BF16)  # X^T [h, i]
            qt = persist.tile([P, ET, N], BF16)  # Q^T [e, i]
            kt = persist.tile([P, ET, N], BF16)  # K^T [e, j]
            vt = persist.tile([P, NT, E], BF16)  # V   [j, e]

            # ---------------- phase 1: load + project ----------------
            with (
                tc.tile_pool(name="wts", bufs=1) as wts,
                tc.tile_pool(name="stage", bufs=6) as stage,
                tc.tile_pool(name="wstage", bufs=4) as wstage,
            ):
                wvb = wts.tile([P, HT, E], BF16, tag="wv")  # Wv natural
                wqb = wts.tile([P, HT, E], BF16, tag="wq")  # Wq natural
                wkb = wts.tile([P, HT, E], BF16, tag="wk")  # Wk natural

                # Engine/queue assignment keeps the three streams from
                # head-of-line-blocking each other:
                #   X:  SWDGE (gpsimd queue) cast-DMA -> xbar (sync queue)
                #   W:  Scalar HWDGE f32 load -> GpSimd engine cast
                #   DVE does only PSUM evacuations.
                def load_x(it):
                    xb = stage.tile([P, H], BF16, tag="ld")
                    nc.gpsimd.dma_start(xb, x[it * P:(it + 1) * P, :])
                    nc.sync.dma_start(xt[:, :, it * P:(it + 1) * P], xb,
                                      transpose=True)

                def load_w(dst, src, ho):
                    ws = wstage.tile([P, E], F32, tag="ws")
                    nc.scalar.dma_start(ws, src[ho * P:(ho + 1) * P, :])
                    nc.gpsimd.tensor_copy(dst[:, ho, :], ws)

                # all X SWDGE issues first so nothing delays the xbar
                # pipeline; W loads run concurrently on the scalar queue.
                for it in range(NT):
                    load_x(it)
                for ho in range(HT):
                    load_w(wvb, wv, ho)
                for ho in range(HT):
                    load_w(wqb, wq, ho)
                for ho in range(HT):
                    load_w(wkb, wk, ho)

                # mask consts (gpsimd engine; only needed by phase 2)
                nc.gpsimd.memset(maskt, 0.0)
                nc.gpsimd.affine_select(
                    out=maskt, in_=maskt, compare_op=mybir.AluOpType.is_ge,
                    fill=NEG, base=0, pattern=[[-1, P]], channel_multiplier=1,
                )
                nc.gpsimd.memset(ones, 1.0)

                def v_chains(jts):
                    # V[j, e] = X Wv   (lhsT = X^T chunk)
                    for jt in jts:
                        for es in range(E // 512):
                            mm = psMM.tile([P, 512], F32, tag="mm")
                            for ho in range(HT):
                                nc.tensor.matmul(
                                    mm,
                                    lhsT=xt[:, ho, jt * P:(jt + 1) * P],
                                    rhs=wvb[:, ho, es * 512:(es + 1) * 512],
                                    start=(ho == 0), stop=(ho == HT - 1),
                                )
                            nc.vector.tensor_copy(
                                vt[:, jt, es * 512:(es + 1) * 512], mm)

                def proj_block(dst, wb, ns):
                    # dst[e, i] = W^T X^T for one 512-wide i block
                    # (lhsT = W natural)
                    for et in range(ET):
                        mm = psMM.tile([P, 512], F32, tag="mm")
                        for ho in range(HT):
                            nc.tensor.matmul(
                                mm,
                                lhsT=wb[:, ho, et * P:(et + 1) * P],
                                rhs=xt[:, ho, ns * 512:(ns + 1) * 512],
                                start=(ho == 0), stop=(ho == HT - 1),
                            )
                        nc.vector.tensor_copy(
                            dst[:, et, ns * 512:(ns + 1) * 512], mm)

                # interleave V chunk-groups with QT ns-blocks to match
                # input arrival order (X chunks stream in at ~4us apiece;
                # wq lands ~40us in, wk ~65us in)
                v_chains(range(0, 6))
                proj_block(qt, wqb, 0)
                v_chains(range(6, 10))
                proj_block(qt, wqb, 1)
                v_chains(range(10, 14))
                proj_block(qt, wqb, 2)
                v_chains(range(14, 16))
                proj_block(qt, wqb, 3)
                for ns in range(N // 512):
                    proj_block(kt, wkb, ns)

            # ---------------- phase 2: attention ----------------
            with (
                tc.tile_pool(name="ptp", bufs=18) as ptp,
                tc.tile_pool(name="obp", bufs=4) as obp,
                tc.tile_pool(name="rip", bufs=2) as rip,
            ):
                for ib in range(NIB):
                    i0 = ib * IB
                    ptlist = {}
                    # S^T[j, i-block] tiles, exp -> P^T
                    for j128 in range(4 * ib, NT):
                        t = j128 - 4 * ib
                        # width of the kept+diagonal region of this tile
                        w = min((t + 1) * P, IB)
                        sp = psMM.tile([P, 512], F32, tag="mm")
                        for et in range(ET):
                            nc.tensor.matmul(
                                sp[:, :w],
                                lhsT=kt[:, et, j128 * P:(j128 + 1) * P],
                                rhs=qt[:, et, i0:i0 + w],
                                start=(et == 0), stop=(et == ET - 1),
                            )
                        if t < 4:
                            # diagonal 128x128 sub-block gets the tri mask
                            nc.vector.tensor_add(
                                sp[:, t * P:(t + 1) * P],
                                sp[:, t * P:(t + 1) * P], maskt)
                        pt = ptp.tile([P, IB], BF16, tag="pt")
                        nc.scalar.activation(
                            pt[:, :w], sp[:, :w],
                            mybir.ActivationFunctionType.Exp,
                            bias=0.0, scale=SCALE,
                        )
                        ptlist[j128] = pt

                    # O[i, :] = P V per 128-row block; row sums via ones-MM
                    for itl in range(4):
                        it = 4 * ib + itl
                        off = itl * P
                        o0 = psO.tile([P, 512], F32, tag="o")
                        o1 = psO.tile([P, 512], F32, tag="o")
                        rs = psR.tile([P, 1], F32, tag="rs")
                        js = list(range(it, NT))
                        for m, j in enumerate(js):
                            pt = ptlist[j]
                            st = (m == 0)
                            en = (m == len(js) - 1)
                            # order (o0, rs, o1): the tiny rs matmul sits
                            # between two 512-wide streams so the next
                            # iteration's LDWEIGHTS hides under o1
                            nc.tensor.matmul(
                                o0, lhsT=pt[:, off:off + P],
                                rhs=vt[:, j, 0:512], start=st, stop=en)
                            nc.tensor.matmul(
                                rs, lhsT=pt[:, off:off + P],
                                rhs=ones, start=st, stop=en)
                            nc.tensor.matmul(
                                o1, lhsT=pt[:, off:off + P],
                                rhs=vt[:, j, 512:1024], start=st, stop=en)
                        ri = rip.tile([P, 1], F32, tag="ri")
                        nc.vector.reciprocal(ri, rs)
                        for half, op in ((0, o0), (1, o1)):
                            ob = obp.tile([P, 512], F32, tag="ob")
                            nc.scalar.mul(ob, op, ri)
                            nc.sync.dma_start(
                                out[it * P:(it + 1) * P,
                                    half * 512:(half + 1) * 512], ob)

    nc.finalize()
    return nc


_NC = None


def _get_nc():
    global _NC
    if _NC is None:
        _NC = build_graph()
    return _NC


def _run(inputs, trace=False, **kwargs):
    x = np.ascontiguousarray(np.asarray(inputs["input"], dtype=np.float32))
    k = np.ascontiguousarray(np.asarray(inputs["k"], dtype=np.float32))
    q = np.ascontiguousarray(np.asarray(inputs["q"], dtype=np.float32))
    v = np.ascontiguousarray(np.asarray(inputs["v"], dtype=np.float32))
    assert x.shape == (B, N, H)
    nc = _get_nc()
    in_maps = [
        {"input": x[b], "k": k, "q": q, "v": v} for b in range(B)
    ]
    res = bass_utils.run_bass_kernel_spmd(
        nc, in_maps, core_ids=list(range(B)), trace=trace, **kwargs)
    outs = np.stack([np.asarray(r["out"]) for r in res.results], axis=0)
    return outs.astype(np.float32), res


def kernel(**inputs):
    outs, _ = _run(inputs, trace=False)
    return outs


# revision 10
# speedup vs baseline: 1.2451x; 1.2451x over previous
"""Distributed Trainium2 kernel for nn_AttentionHead (B=8, N=2048, H=E=1024).

Single attention head with an UPPER-triangular mask (reference masks i > j,
i.e. position i attends to j >= i), softmax over j, applied per batch:

    K = X Wk; Q = X Wq; V = X Wv
    S = Q K^T / sqrt(E);  S[i, j] = -inf for i > j
    O = softmax_j(S) V

Sharding: pure data parallel -- batch b (8) maps 1:1 onto the 8 NeuronCores.
Weights replicated; no collectives.

Per-core algorithm (v3; all matmuls bf16 with fp32 PSUM accumulation):
  - X^T is produced by the DMA xbar (one DMA_TRANSPOSE per [128, 1024]
    chunk with a 3D output AP, all on the Sync HWDGE queue -- concurrent
    xbar ops on both HWDGE queues corrupt data). f32->bf16 casts happen
    inside the SWDGE load DMAs. The PE does nothing but matmuls.
  - Projections Q^T[e,i], K^T[e,j], V[j,e] use the weights in NATURAL
    layout as lhsT (no weight transposes at all):
        qt = matmul(lhsT=Wq[h, e-slice], rhs=X^T[h, i])   etc.
  - The attention phase is computed TRANSPOSED: S^T tiles [128 j, 512 i]
    from matmul(lhsT=K^T, rhs=Q^T); exp writes P^T which is exactly the
    lhsT the PV matmul needs (no per-tile PE transposes of P). Row sums
    fall out of a [128,1] ones-matmul sharing the PV stationary weights.
  - Triangular structure is skipped at 128-col granularity on both the
    S^T and PV sides; the diagonal 128x128 gets an additive -1e30 mask.
"""

import numpy as np

try:
    import concourse.bass as bass
except ImportError:  # fresh grading dir: concourse comes from the site repo
    import sys

    for p in ("/opt/trn_rl_repo", "/root/.axon_site/_ro/trn_rl_repo"):
        if p not in sys.path:
            sys.path.append(p)
    import concourse.bass as bass

import concourse.mybir as mybir
import concourse.tile as tile
from concourse import bacc, bass_utils
from concourse.masks import make_identity

B, N, H, E = 8, 2048, 1024, 1024
P = 128
HT = H // P  # 8 h-tiles
ET = E // P  # 8 e-tiles
NT = N // P  # 16 row tiles
IB = 512  # i-block width in the attention phase
NIB = N // IB  # 4
F32 = mybir.dt.float32
BF16 = mybir.dt.bfloat16
SCALE = 1.0 / float(np.sqrt(E))
NEG = -1.0e30


def build_graph():
    nc = bacc.Bacc("TRN2", target_bir_lowering=False, debug=False,
                   enable_asserts=False)
    x = nc.dram_tensor("input", [N, H], F32, kind="ExternalInput").ap()
    wk = nc.dram_tensor("k", [H, E], F32, kind="ExternalInput").ap()
    wq = nc.dram_tensor("q", [H, E], F32, kind="ExternalInput").ap()
    wv = nc.dram_tensor("v", [H, E], F32, kind="ExternalInput").ap()
    out = nc.dram_tensor("out", [N, E], F32, kind="ExternalOutput").ap()

    with tile.TileContext(nc) as tc:
        with (
            tc.tile_pool(name="const", bufs=1) as constp,
            tc.tile_pool(name="persist", bufs=1) as persist,
            tc.tile_pool(name="psMM", bufs=3, space="PSUM") as psMM,
        ):
            maskt = constp.tile([P, P], F32)
            ones = constp.tile([P, 1], BF16)
            ident16 = constp.tile([P, P], BF16)

            xt = persist.tile([P, HT, N], BF16)  # X^T [h, i]
            qt = persist.tile([P, ET, N], BF16)  # Q^T [e, i]
            kt = persist.tile([P, ET, N], BF16)  # K^T [e, j]
            vt = persist.tile([P, NT, E], BF16)  # V   [j, e]

            # ---------------- phase 1: load + project ----------------
            with (
                tc.tile_pool(name="wts", bufs=1) as wts,
                tc.tile_pool(name="stage", bufs=6) as stage,
                tc.tile_pool(name="wstage", bufs=4) as wstage,
                tc.tile_pool(name="psT", bufs=3, space="PSUM") as psT,
            ):
                wvb = wts.tile([P, HT, E], BF16, tag="wv")  # Wv natural
                wqb = wts.tile([P, HT, E], BF16, tag="wq")  # Wq natural
                wkb = wts.tile([P, HT, E], BF16, tag="wk")  # Wk natural

                # Engine/queue assignment keeps the three streams from
                # head-of-line-blocking each other:
                #   X:  SWDGE (gpsimd queue) cast-DMA, PE transpose
                #   Wv: Scalar HWDGE f32 load + DVE cast (needed first)
                #   Wq/Wk: Scalar HWDGE f32 load + GpSimd cast
                #   DVE otherwise does only PSUM evacuations.
                xbs = {}

                def load_x(it):
                    xb = stage.tile([P, H], BF16, tag="ld")
                    nc.gpsimd.dma_start(xb, x[it * P:(it + 1) * P, :])
                    xbs[it] = xb

                def load_w(dst, src, ho, cast_eng):
                    ws = wstage.tile([P, E], F32, tag="ws")
                    nc.scalar.dma_start(ws, src[ho * P:(ho + 1) * P, :])
                    cast_eng.tensor_copy(dst[:, ho, :], ws)

                make_identity(nc, ident16)
                for it in range(NT):
                    load_x(it)
                for ho in range(HT):
                    load_w(wvb, wv, ho, nc.vector)
                for ho in range(HT):
                    load_w(wqb, wq, ho, nc.gpsimd)
                for ho in range(HT):
                    load_w(wkb, wk, ho, nc.gpsimd)

                # mask consts (gpsimd engine; only needed by phase 2)
                nc.gpsimd.memset(maskt, 0.0)
                nc.gpsimd.affine_select(
                    out=maskt, in_=maskt, compare_op=mybir.AluOpType.is_ge,
                    fill=NEG, base=0, pattern=[[-1, P]], channel_multiplier=1,
                )
                nc.gpsimd.memset(ones, 1.0)

                def transpose_chunk(it):
                    xb = xbs.pop(it)
                    for ho in range(HT):
                        tp = psT.tile([P, P], BF16, tag="tp")
                        nc.tensor.transpose(
                            tp, xb[:, ho * P:(ho + 1) * P], ident16)
                        nc.vector.tensor_copy(
                            xt[:, ho, it * P:(it + 1) * P], tp)

                def v_chains(jts):
                    # V[j, e] = X Wv   (lhsT = X^T chunk)
                    for jt in jts:
                        for es in range(E // 512):
                            mm = psMM.tile([P, 512], F32, tag="mm")
                            for ho in range(HT):
                                nc.tensor.matmul(
                                    mm,
                                    lhsT=xt[:, ho, jt * P:(jt + 1) * P],
                                    rhs=wvb[:, ho, es * 512:(es + 1) * 512],
                                    start=(ho == 0), stop=(ho == HT - 1),
                                )
                            nc.vector.tensor_copy(
                                vt[:, jt, es * 512:(es + 1) * 512], mm)

                def proj_block(dst, wb, ns):
                    # dst[e, i] = W^T X^T for one 512-wide i block
                    # (lhsT = W natural)
                    for et in range(ET):
                        mm = psMM.tile([P, 512], F32, tag="mm")
                        for ho in range(HT):
                            nc.tensor.matmul(
                                mm,
                                lhsT=wb[:, ho, et * P:(et + 1) * P],
                                rhs=xt[:, ho, ns * 512:(ns + 1) * 512],
                                start=(ho == 0), stop=(ho == HT - 1),
                            )
                        nc.vector.tensor_copy(
                            dst[:, et, ns * 512:(ns + 1) * 512], mm)

                # PE order: transposes first (they only need X chunks, so
                # the PE has work ~5us in), V chains as soon as wv lands,
                # then the rest, transposes interleaved one chunk ahead.
                for it in range(6):
                    transpose_chunk(it)
                for it in range(6, NT):
                    v_chains([it - 6])
                    transpose_chunk(it)
                v_chains(range(NT - 6, NT))
                for ns in range(N // 512):
                    proj_block(qt, wqb, ns)
                for ns in range(N // 512):
                    proj_block(kt, wkb, ns)

            # ---------------- phase 2: attention ----------------
            with (
                tc.tile_pool(name="ptp", bufs=18) as ptp,
                tc.tile_pool(name="obp", bufs=4) as obp,
                tc.tile_pool(name="rip", bufs=2) as rip,
                tc.tile_pool(name="psO", bufs=4, space="PSUM") as psO,
                tc.tile_pool(name="psR", bufs=1, space="PSUM") as psR,
            ):
                for ib in range(NIB):
                    i0 = ib * IB
                    ptlist = {}
                    # S^T[j, i-block] tiles, exp -> P^T
                    for j128 in range(4 * ib, NT):
                        t = j128 - 4 * ib
                        # width of the kept+diagonal region of this tile
                        w = min((t + 1) * P, IB)
                        sp = psMM.tile([P, 512], F32, tag="mm")
                        for et in range(ET):
                            nc.tensor.matmul(
                                sp[:, :w],
                                lhsT=kt[:, et, j128 * P:(j128 + 1) * P],
                                rhs=qt[:, et, i0:i0 + w],
                                start=(et == 0), stop=(et == ET - 1),
                            )
                        if t < 4:
                            # diagonal 128x128 sub-block gets the tri mask
                            nc.vector.tensor_add(
                                sp[:, t * P:(t + 1) * P],
                                sp[:, t * P:(t + 1) * P], maskt)
                        pt = ptp.tile([P, IB], BF16, tag="pt")
                        nc.scalar.activation(
                            pt[:, :w], sp[:, :w],
                            mybir.ActivationFunctionType.Exp,
                            bias=0.0, scale=SCALE,
                        )
                        ptlist[j128] = pt

                    # O[i, :] = P V per 128-row block; row sums via ones-MM
                    for itl in range(4):
                        it = 4 * ib + itl
                        off = itl * P
                        o0 = psO.tile([P, 512], F32, tag="o")
                        o1 = psO.tile([P, 512], F32, tag="o")
                        rs = psR.tile([P, 1], F32, tag="rs")
                        js = list(range(it, NT))
                        for m, j in enumerate(js):
                            pt = ptlist[j]
                            st = (m == 0)
                            en = (m == len(js) - 1)
                            # order (o0, rs, o1): the tiny rs matmul sits
                            # between two 512-wide streams so the next
                            # iteration's LDWEIGHTS hides under o1
                            nc.tensor.matmul(
                                o0, lhsT=pt[:, off:off + P],
                                rhs=vt[:, j, 0:512], start=st, stop=en)
                            nc.tensor.matmul(
                                rs, lhsT=pt[:, off:off + P],
                                rhs=ones, start=st, stop=en)
                            nc.tensor.matmul(
                                o1, lhsT=pt[:, off:off + P],
                                rhs=vt[:, j, 512:1024], start=st, stop=en)
                        ri = rip.tile([P, 1], F32, tag="ri")
                        nc.vector.reciprocal(ri, rs)
                        for half, op in ((0, o0), (1, o1)):
                            ob = obp.tile([P, 512], F32, tag="ob")
                            nc.scalar.mul(ob, op, ri)
                            nc.sync.dma_start(
                                out[it * P:(it + 1) * P,
                                    half * 512:(half + 1) * 512], ob)

    nc.finalize()
    return nc


_NC = None


def _get_nc():
    global _NC
    if _NC is None:
        _NC = build_graph()
    return _NC


def _run(inputs, trace=False, **kwargs):
    x = np.ascontiguousarray(np.asarray(inputs["input"], dtype=np.float32))
    k = np.ascontiguousarray(np.asarray(inputs["k"], dtype=np.float32))
    q = np.ascontiguousarray(np.asarray(inputs["q"], dtype=np.float32))
    v = np.ascontiguousarray(np.asarray(inputs["v"], dtype=np.float32))
    assert x.shape == (B, N, H)
    nc = _get_nc()
    in_maps = [
        {"input": x[b], "k": k, "q": q, "v": v} for b in range(B)
    ]
    res = bass_utils.run_bass_kernel_spmd(
        nc, in_maps, core_ids=list(range(B)), trace=trace, **kwargs)
    outs = np.stack([np.asarray(r["out"]) for r in res.results], axis=0)
    return outs.astype(np.float32), res


def kernel(**inputs):
    outs, _ = _run(inputs, trace=False)
    return outs


# revision 12
# speedup vs baseline: 1.2476x; 1.0020x over previous
"""Distributed Trainium2 kernel for nn_AttentionHead (B=8, N=2048, H=E=1024).

Single attention head with an UPPER-triangular mask (reference masks i > j,
i.e. position i attends to j >= i), softmax over j, applied per batch:

    K = X Wk; Q = X Wq; V = X Wv
    S = Q K^T / sqrt(E);  S[i, j] = -inf for i > j
    O = softmax_j(S) V

Sharding: pure data parallel -- batch b (8) maps 1:1 onto the 8 NeuronCores.
Weights replicated; no collectives.

Per-core algorithm (v3; all matmuls bf16 with fp32 PSUM accumulation):
  - X^T is produced by the DMA xbar (one DMA_TRANSPOSE per [128, 1024]
    chunk with a 3D output AP, all on the Sync HWDGE queue -- concurrent
    xbar ops on both HWDGE queues corrupt data). f32->bf16 casts happen
    inside the SWDGE load DMAs. The PE does nothing but matmuls.
  - Projections Q^T[e,i], K^T[e,j], V[j,e] use the weights in NATURAL
    layout as lhsT (no weight transposes at all):
        qt = matmul(lhsT=Wq[h, e-slice], rhs=X^T[h, i])   etc.
  - The attention phase is computed TRANSPOSED: S^T tiles [128 j, 512 i]
    from matmul(lhsT=K^T, rhs=Q^T); exp writes P^T which is exactly the
    lhsT the PV matmul needs (no per-tile PE transposes of P). Row sums
    fall out of a [128,1] ones-matmul sharing the PV stationary weights.
  - Triangular structure is skipped at 128-col granularity on both the
    S^T and PV sides; the diagonal 128x128 gets an additive -1e30 mask.
"""

import numpy as np

try:
    import concourse.bass as bass
except ImportError:  # fresh grading dir: concourse comes from the site repo
    import sys

    for p in ("/opt/trn_rl_repo", "/root/.axon_site/_ro/trn_rl_repo"):
        if p not in sys.path:
            sys.path.append(p)
    import concourse.bass as bass

import concourse.mybir as mybir
import concourse.tile as tile
from concourse import bacc, bass_utils
from concourse.masks import make_identity

B, N, H, E = 8, 2048, 1024, 1024
P = 128
HT = H // P  # 8 h-tiles
ET = E // P  # 8 e-tiles
NT = N // P  # 16 row tiles
IB = 512  # i-block width in the attention phase
NIB = N // IB  # 4
F32 = mybir.dt.float32
BF16 = mybir.dt.bfloat16
SCALE = 1.0 / float(np.sqrt(E))
NEG = -1.0e30


def build_graph():
    nc = bacc.Bacc("TRN2", target_bir_lowering=False, debug=False,
                   enable_asserts=False)
    x = nc.dram_tensor("input", [N, H], F32, kind="ExternalInput").ap()
    wk = nc.dram_tensor("k", [H, E], F32, kind="ExternalInput").ap()
    wq = nc.dram_tensor("q", [H, E], F32, kind="ExternalInput").ap()
    wv = nc.dram_tensor("v", [H, E], F32, kind="ExternalInput").ap()
    out = nc.dram_tensor("out", [N, E], F32, kind="ExternalOutput").ap()

    with tile.TileContext(nc) as tc:
        with (
            tc.tile_pool(name="const", bufs=1) as constp,
            tc.tile_pool(name="persist", bufs=1) as persist,
            tc.tile_pool(name="psMM", bufs=3, space="PSUM") as psMM,
        ):
            maskt = constp.tile([P, P], F32)
            ones = constp.tile([P, 1], BF16)
            ident16 = constp.tile([P, P], BF16)

            xt = persist.tile([P, HT, N], BF16)  # X^T [h, i]
            qt = persist.tile([P, ET, N], BF16)  # Q^T [e, i]
            kt = persist.tile([P, ET, N], BF16)  # K^T [e, j]
            vt = persist.tile([P, NT, E], BF16)  # V   [j, e]

            # ---------------- phase 1: load + project ----------------
            with (
                tc.tile_pool(name="wts", bufs=1) as wts,
                tc.tile_pool(name="stage", bufs=6) as stage,
                tc.tile_pool(name="wstage", bufs=4) as wstage,
                tc.tile_pool(name="psT", bufs=3, space="PSUM") as psT,
            ):
                wvb = wts.tile([P, HT, E], BF16, tag="wv")  # Wv natural
                wqb = wts.tile([P, HT, E], BF16, tag="wq")  # Wq natural
                wkb = wts.tile([P, HT, E], BF16, tag="wk")  # Wk natural

                # Engine/queue assignment keeps the three streams from
                # head-of-line-blocking each other:
                #   X:  SWDGE (gpsimd queue) cast-DMA, PE transpose
                #   Wv: Scalar HWDGE f32 load + DVE cast (needed first)
                #   Wq/Wk: Scalar HWDGE f32 load + GpSimd cast
                #   DVE otherwise does only PSUM evacuations.
                xbs = {}
                N_PE_T = 6  # chunks 0..5 transposed on the PE (early
                # deadlines + fills the head); the rest go via the xbar
                # on the idle sync queue (late deadlines, zero PE cost)

                def load_x(it):
                    xb = stage.tile([P, H], BF16, tag="ld")
                    nc.gpsimd.dma_start(xb, x[it * P:(it + 1) * P, :])
                    if it >= N_PE_T:
                        nc.sync.dma_start(
                            xt[:, :, it * P:(it + 1) * P], xb,
                            transpose=True)
                    else:
                        xbs[it] = xb

                def load_w(dst, src, ho, cast_eng):
                    ws = wstage.tile([P, E], F32, tag="ws")
                    nc.scalar.dma_start(ws, src[ho * P:(ho + 1) * P, :])
                    cast_eng.tensor_copy(dst[:, ho, :], ws)

                make_identity(nc, ident16)
                for it in range(NT):
                    load_x(it)
                for ho in range(HT):
                    load_w(wvb, wv, ho, nc.vector)
                for ho in range(HT):
                    load_w(wqb, wq, ho, nc.gpsimd)
                for ho in range(HT):
                    load_w(wkb, wk, ho, nc.gpsimd)

                # mask consts (gpsimd engine; only needed by phase 2)
                nc.gpsimd.memset(maskt, 0.0)
                nc.gpsimd.affine_select(
                    out=maskt, in_=maskt, compare_op=mybir.AluOpType.is_ge,
                    fill=NEG, base=0, pattern=[[-1, P]], channel_multiplier=1,
                )
                nc.gpsimd.memset(ones, 1.0)

                def transpose_chunk(it):
                    xb = xbs.pop(it)
                    for ho in range(HT):
                        tp = psT.tile([P, P], BF16, tag="tp")
                        nc.tensor.transpose(
                            tp, xb[:, ho * P:(ho + 1) * P], ident16)
                        nc.vector.tensor_copy(
                            xt[:, ho, it * P:(it + 1) * P], tp)

                def v_chains(jts):
                    # V[j, e] = X Wv   (lhsT = X^T chunk)
                    for jt in jts:
                        for es in range(E // 512):
                            mm = psMM.tile([P, 512], F32, tag="mm")
                            for ho in range(HT):
                                nc.tensor.matmul(
                                    mm,
                                    lhsT=xt[:, ho, jt * P:(jt + 1) * P],
                                    rhs=wvb[:, ho, es * 512:(es + 1) * 512],
                                    start=(ho == 0), stop=(ho == HT - 1),
                                )
                            nc.vector.tensor_copy(
                                vt[:, jt, es * 512:(es + 1) * 512], mm)

                def proj_block(dst, wb, ns):
                    # dst[e, i] = W^T X^T for one 512-wide i block
                    # (lhsT = W natural)
                    for et in range(ET):
                        mm = psMM.tile([P, 512], F32, tag="mm")
                        for ho in range(HT):
                            nc.tensor.matmul(
                                mm,
                                lhsT=wb[:, ho, et * P:(et + 1) * P],
                                rhs=xt[:, ho, ns * 512:(ns + 1) * 512],
                                start=(ho == 0), stop=(ho == HT - 1),
                            )
                        nc.vector.tensor_copy(
                            dst[:, et, ns * 512:(ns + 1) * 512], mm)

                # PE order: the six PE transposes first (they only need X
                # chunks, so the PE has work ~5us in), then V chains as wv
                # lands, then the projections.
                for it in range(N_PE_T):
                    transpose_chunk(it)
                v_chains(range(NT))
                for ns in range(N // 512):
                    proj_block(qt, wqb, ns)
                for ns in range(N // 512):
                    proj_block(kt, wkb, ns)

            # ---------------- phase 2: attention ----------------
            with (
                tc.tile_pool(name="ptp", bufs=18) as ptp,
                tc.tile_pool(name="obp", bufs=4) as obp,
                tc.tile_pool(name="rip", bufs=2) as rip,
                tc.tile_pool(name="psO", bufs=4, space="PSUM") as psO,
                tc.tile_pool(name="psR", bufs=1, space="PSUM") as psR,
            ):
                for ib in range(NIB):
                    i0 = ib * IB
                    ptlist = {}
                    # S^T[j, i-block] tiles, exp -> P^T
                    for j128 in range(4 * ib, NT):
                        t = j128 - 4 * ib
                        # width of the kept+diagonal region of this tile
                        w = min((t + 1) * P, IB)
                        sp = psMM.tile([P, 512], F32, tag="mm")
                        for et in range(ET):
                            nc.tensor.matmul(
                                sp[:, :w],
                                lhsT=kt[:, et, j128 * P:(j128 + 1) * P],
                                rhs=qt[:, et, i0:i0 + w],
                                start=(et == 0), stop=(et == ET - 1),
                            )
                        if t < 4:
                            # diagonal 128x128 sub-block gets the tri mask
                            nc.vector.tensor_add(
                                sp[:, t * P:(t + 1) * P],
                                sp[:, t * P:(t + 1) * P], maskt)
                        pt = ptp.tile([P, IB], BF16, tag="pt")
                        nc.scalar.activation(
                            pt[:, :w], sp[:, :w],
                            mybir.ActivationFunctionType.Exp,
                            bias=0.0, scale=SCALE,
                        )
                        ptlist[j128] = pt

                    # O[i, :] = P V per 128-row block; row sums via ones-MM
                    for itl in range(4):
                        it = 4 * ib + itl
                        off = itl * P
                        o0 = psO.tile([P, 512], F32, tag="o")
                        o1 = psO.tile([P, 512], F32, tag="o")
                        rs = psR.tile([P, 1], F32, tag="rs")
                        js = list(range(it, NT))
                        for m, j in enumerate(js):
                            pt = ptlist[j]
                            st = (m == 0)
                            en = (m == len(js) - 1)
                            # order (o0, rs, o1): the tiny rs matmul sits
                            # between two 512-wide streams so the next
                            # iteration's LDWEIGHTS hides under o1
                            nc.tensor.matmul(
                                o0, lhsT=pt[:, off:off + P],
                                rhs=vt[:, j, 0:512], start=st, stop=en)
                            nc.tensor.matmul(
                                rs, lhsT=pt[:, off:off + P],
                                rhs=ones, start=st, stop=en)
                            nc.tensor.matmul(
                                o1, lhsT=pt[:, off:off + P],
                                rhs=vt[:, j, 512:1024], start=st, stop=en)
                        ri = rip.tile([P, 1], F32, tag="ri")
                        nc.vector.reciprocal(ri, rs)
                        for half, op in ((0, o0), (1, o1)):
                            ob = obp.tile([P, 512], F32, tag="ob")
                            nc.scalar.mul(ob, op, ri)
                            nc.sync.dma_start(
                                out[it * P:(it + 1) * P,
                                    half * 512:(half + 1) * 512], ob)

    nc.finalize()
    return nc


_NC = None


def _get_nc():
    global _NC
    if _NC is None:
        _NC = build_graph()
    return _NC


def _run(inputs, trace=False, **kwargs):
    x = np.ascontiguousarray(np.asarray(inputs["input"], dtype=np.float32))
    k = np.ascontiguousarray(np.asarray(inputs["k"], dtype=np.float32))
    q = np.ascontiguousarray(np.asarray(inputs["q"], dtype=np.float32))
    v = np.ascontiguousarray(np.asarray(inputs["v"], dtype=np.float32))
    assert x.shape == (B, N, H)
    nc = _get_nc()
    in_maps = [
        {"input": x[b], "k": k, "q": q, "v": v} for b in range(B)
    ]
    res = bass_utils.run_bass_kernel_spmd(
        nc, in_maps, core_ids=list(range(B)), trace=trace, **kwargs)
    outs = np.stack([np.asarray(r["out"]) for r in res.results], axis=0)
    return outs.astype(np.float32), res


def kernel(**inputs):
    outs, _ = _run(inputs, trace=False)
    return outs


# revision 15
# speedup vs baseline: 1.2951x; 1.0381x over previous
"""Distributed Trainium2 kernel for nn_AttentionHead (B=8, N=2048, H=E=1024).

Single attention head with an UPPER-triangular mask (reference masks i > j,
i.e. position i attends to j >= i), softmax over j, applied per batch:

    K = X Wk; Q = X Wq; V = X Wv
    S = Q K^T / sqrt(E);  S[i, j] = -inf for i > j
    O = softmax_j(S) V

Sharding: pure data parallel -- batch b (8) maps 1:1 onto the 8 NeuronCores.
Weights replicated; no collectives.

Per-core algorithm (v8; all matmuls bf16 with fp32 PSUM accumulation):
  - Score side folds both projections into one: S = X A X^T with
    A = Wq Wk^T, G^T = A^T X^T, so S^T tiles come from
    matmul(lhsT=X^T, rhs=G^T) -- X^T itself is the K-side operand.
  - All layout transposes (X chunks, Wq, Wk) run on the PE against a
    bf16 identity; pipelined they cost ~80ns apiece, interleaved into
    the matmul stream so HAM stays warm.
  - The attention phase is computed TRANSPOSED: exp writes P^T which is
    exactly the lhsT the PV matmul needs (no per-tile transposes of P).
    Row sums fall out of a [128,1] ones-matmul sharing the PV stationary
    weights, ordered (o0, rs, o1) so the next LDWEIGHTS hides under a
    512-wide stream.
  - Triangular structure is skipped at 128-col granularity on both the
    S^T and PV sides; the diagonal 128x128 gets an additive -1e30 mask.
  - Engine/queue split: X loads on SWDGE with in-DMA f32->bf16 cast;
    W loads f32 on the Scalar HWDGE queue; wv casts on DVE (early
    deadline), wq/wk casts on GpSimd; DVE otherwise only evacuates PSUM.
"""

import numpy as np

try:
    import concourse.bass as bass
except ImportError:  # fresh grading dir: concourse comes from the site repo
    import sys

    for p in ("/opt/trn_rl_repo", "/root/.axon_site/_ro/trn_rl_repo"):
        if p not in sys.path:
            sys.path.append(p)
    import concourse.bass as bass

import concourse.mybir as mybir
import concourse.tile as tile
from concourse import bacc, bass_utils
from concourse.masks import make_identity

B, N, H, E = 8, 2048, 1024, 1024
P = 128
HT = H // P  # 8 h-tiles
ET = E // P  # 8 e-tiles
NT = N // P  # 16 row tiles
IB = 512  # i-block width in the attention phase
NIB = N // IB  # 4
F32 = mybir.dt.float32
BF16 = mybir.dt.bfloat16
SCALE = 1.0 / float(np.sqrt(E))
NEG = -1.0e30


def build_graph():
    nc = bacc.Bacc("TRN2", target_bir_lowering=False, debug=False,
                   enable_asserts=False)
    x = nc.dram_tensor("input", [N, H], F32, kind="ExternalInput").ap()
    wk = nc.dram_tensor("k", [H, E], F32, kind="ExternalInput").ap()
    wq = nc.dram_tensor("q", [H, E], F32, kind="ExternalInput").ap()
    wv = nc.dram_tensor("v", [H, E], F32, kind="ExternalInput").ap()
    out = nc.dram_tensor("out", [N, E], F32, kind="ExternalOutput").ap()

    with tile.TileContext(nc) as tc:
        with (
            tc.tile_pool(name="const", bufs=1) as constp,
            tc.tile_pool(name="persist", bufs=1) as persist,
            tc.tile_pool(name="psMM", bufs=3, space="PSUM") as psMM,
        ):
            maskt = constp.tile([P, P], F32)
            ones = constp.tile([P, 1], BF16)
            ident16 = constp.tile([P, P], BF16)

            xt = persist.tile([P, HT, N], BF16)  # X^T [h, i]
            gt = persist.tile([P, HT, N], BF16)  # G^T [h2, i], G = X A
            vt = persist.tile([P, NT, E], BF16)  # V   [j, e]

            # ---------------- phase 1: load + project ----------------
            with (
                tc.tile_pool(name="wts", bufs=1) as wts,
                tc.tile_pool(name="stage", bufs=6) as stage,
                tc.tile_pool(name="wstage", bufs=4) as wstage,
                tc.tile_pool(name="wbst", bufs=6) as wbst,
                tc.tile_pool(name="psT", bufs=3, space="PSUM") as psT,
            ):
                wvb = wts.tile([P, HT, E], BF16, tag="wv")   # Wv natural
                wqT = wts.tile([P, ET, H], BF16, tag="wqT")  # Wq^T [e, h]
                wkT = wts.tile([P, ET, H], BF16, tag="wkT")  # Wk^T [e, h]
                ab = wts.tile([P, HT, H], BF16, tag="A")     # A [h1, h2]

                xbs = {}
                wbs = {}

                def load_x(it):
                    xb = stage.tile([P, H], BF16, tag="ld")
                    nc.gpsimd.dma_start(xb, x[it * P:(it + 1) * P, :])
                    xbs[it] = xb

                def load_wv(ho):
                    ws = wstage.tile([P, E], F32, tag="ws")
                    nc.scalar.dma_start(ws, wv[ho * P:(ho + 1) * P, :])
                    nc.vector.tensor_copy(wvb[:, ho, :], ws)

                def load_wqk(src, key, ho):
                    ws = wstage.tile([P, E], F32, tag="ws")
                    nc.scalar.dma_start(ws, src[ho * P:(ho + 1) * P, :])
                    wb = wbst.tile([P, E], BF16, tag="wb")
                    nc.gpsimd.tensor_copy(wb, ws)
                    wbs[(key, ho)] = wb

                make_identity(nc, ident16)
                for it in range(NT):
                    load_x(it)
                for ho in range(HT):
                    load_wv(ho)
                for ho in range(HT):
                    load_wqk(wq, "q", ho)
                for ho in range(HT):
                    load_wqk(wk, "k", ho)

                # mask consts (gpsimd engine; only needed by phase 2)
                nc.gpsimd.memset(maskt, 0.0)
                nc.gpsimd.affine_select(
                    out=maskt, in_=maskt, compare_op=mybir.AluOpType.is_ge,
                    fill=NEG, base=0, pattern=[[-1, P]], channel_multiplier=1,
                )
                nc.gpsimd.memset(ones, 1.0)

                def transpose_chunk(src_tile, dst, col):
                    # 8 PE transposes: src [128, 1024] -> dst[:, :, col128]
                    for ho in range(HT):
                        tp = psT.tile([P, P], BF16, tag="tp")
                        nc.tensor.transpose(
                            tp, src_tile[:, ho * P:(ho + 1) * P], ident16)
                        nc.vector.tensor_copy(
                            dst[:, ho, col * P:(col + 1) * P], tp)

                def v_chains(jts):
                    # V[j, e] = X Wv   (lhsT = X^T chunk)
                    for jt in jts:
                        for es in range(E // 512):
                            mm = psMM.tile([P, 512], F32, tag="mm")
                            for ho in range(HT):
                                nc.tensor.matmul(
                                    mm,
                                    lhsT=xt[:, ho, jt * P:(jt + 1) * P],
                                    rhs=wvb[:, ho, es * 512:(es + 1) * 512],
                                    start=(ho == 0), stop=(ho == HT - 1),
                                )
                            nc.vector.tensor_copy(
                                vt[:, jt, es * 512:(es + 1) * 512], mm)

                # PE order: X transposes first (they only need X chunks, so
                # the PE has work ~5us in), V chains as wv lands with the
                # remaining X and W transposes interleaved between them.
                for it in range(6):
                    transpose_chunk(xbs.pop(it), xt, it)
                for it in range(6, NT):
                    v_chains([it - 6])
                    transpose_chunk(xbs.pop(it), xt, it)
                for ho in range(4):
                    v_chains([10 + ho])
                    transpose_chunk(wbs.pop(("q", 2 * ho)), wqT, 2 * ho)
                    transpose_chunk(wbs.pop(("q", 2 * ho + 1)), wqT,
                                    2 * ho + 1)
                for ho in range(2):
                    v_chains([14 + ho])
                    for k in range(4):
                        transpose_chunk(wbs.pop(("k", 4 * ho + k)), wkT,
                                        4 * ho + k)

                # ---- A[h1, h2] = Wq Wk^T ----
                for h1t in range(HT):
                    for h2s in range(H // 512):
                        mm = psMM.tile([P, 512], F32, tag="mm")
                        for et in range(ET):
                            nc.tensor.matmul(
                                mm,
                                lhsT=wqT[:, et, h1t * P:(h1t + 1) * P],
                                rhs=wkT[:, et, h2s * 512:(h2s + 1) * 512],
                                start=(et == 0), stop=(et == ET - 1),
                            )
                        nc.vector.tensor_copy(
                            ab[:, h1t, h2s * 512:(h2s + 1) * 512], mm)

                # ---- GT[h2, i] = (X A)^T = A^T X^T ----
                for ns in range(N // 512):
                    for h2t in range(HT):
                        mm = psMM.tile([P, 512], F32, tag="mm")
                        for h1t in range(HT):
                            nc.tensor.matmul(
                                mm,
                                lhsT=ab[:, h1t, h2t * P:(h2t + 1) * P],
                                rhs=xt[:, h1t, ns * 512:(ns + 1) * 512],
                                start=(h1t == 0), stop=(h1t == HT - 1),
                            )
                        nc.vector.tensor_copy(
                            gt[:, h2t, ns * 512:(ns + 1) * 512], mm)

            # ---------------- phase 2: attention ----------------
            with (
                tc.tile_pool(name="ptp", bufs=18) as ptp,
                tc.tile_pool(name="obp", bufs=4) as obp,
                tc.tile_pool(name="rip", bufs=2) as rip,
                tc.tile_pool(name="psO", bufs=4, space="PSUM") as psO,
                tc.tile_pool(name="psR", bufs=1, space="PSUM") as psR,
            ):
                for ib in range(NIB):
                    i0 = ib * IB
                    ptlist = {}
                    # S^T[j, i-block] tiles, exp -> P^T
                    for j128 in range(4 * ib, NT):
                        t = j128 - 4 * ib
                        # width of the kept+diagonal region of this tile
                        w = min((t + 1) * P, IB)
                        sp = psMM.tile([P, 512], F32, tag="mm")
                        for h2t in range(HT):
                            nc.tensor.matmul(
                                sp[:, :w],
                                lhsT=xt[:, h2t, j128 * P:(j128 + 1) * P],
                                rhs=gt[:, h2t, i0:i0 + w],
                                start=(h2t == 0), stop=(h2t == HT - 1),
                            )
                        if t < 4:
                            # diagonal 128x128 sub-block gets the tri mask
                            nc.vector.tensor_add(
                                sp[:, t * P:(t + 1) * P],
                                sp[:, t * P:(t + 1) * P], maskt)
                        pt = ptp.tile([P, IB], BF16, tag="pt")
                        nc.scalar.activation(
                            pt[:, :w], sp[:, :w],
                            mybir.ActivationFunctionType.Exp,
                            bias=0.0, scale=SCALE,
                        )
                        ptlist[j128] = pt

                    # O[i, :] = P V per 128-row block; row sums via ones-MM
                    for itl in range(4):
                        it = 4 * ib + itl
                        off = itl * P
                        o0 = psO.tile([P, 512], F32, tag="o")
                        o1 = psO.tile([P, 512], F32, tag="o")
                        rs = psR.tile([P, 1], F32, tag="rs")
                        js = list(range(it, NT))
                        for m, j in enumerate(js):
                            pt = ptlist[j]
                            st = (m == 0)
                            en = (m == len(js) - 1)
                            # order (o0, rs, o1): the tiny rs matmul sits
                            # between two 512-wide streams so the next
                            # iteration's LDWEIGHTS hides under o1
                            nc.tensor.matmul(
                                o0, lhsT=pt[:, off:off + P],
                                rhs=vt[:, j, 0:512], start=st, stop=en)
                            nc.tensor.matmul(
                                rs, lhsT=pt[:, off:off + P],
                                rhs=ones, start=st, stop=en)
                            nc.tensor.matmul(
                                o1, lhsT=pt[:, off:off + P],
                                rhs=vt[:, j, 512:1024], start=st, stop=en)
                        ri = rip.tile([P, 1], F32, tag="ri")
                        nc.vector.reciprocal(ri, rs)
                        for half, op in ((0, o0), (1, o1)):
                            ob = obp.tile([P, 512], F32, tag="ob")
                            nc.scalar.mul(ob, op, ri)
                            nc.sync.dma_start(
                                out[it * P:(it + 1) * P,
                                    half * 512:(half + 1) * 512], ob)

    nc.finalize()
    return nc


_NC = None


def _get_nc():
    global _NC
    if _NC is None:
        _NC = build_graph()
    return _NC


def _run(inputs, trace=False, **kwargs):
    x = np.ascontiguousarray(np.asarray(inputs["input"], dtype=np.float32))
    k = np.ascontiguousarray(np.asarray(inputs["k"], dtype=np.float32))
    q = np.ascontiguousarray(np.asarray(inputs["q"], dtype=np.float32))
    v = np.ascontiguousarray(np.asarray(inputs["v"], dtype=np.float32))
    assert x.shape == (B, N, H)
    nc = _get_nc()
    in_maps = [
        {"input": x[b], "k": k, "q": q, "v": v} for b in range(B)
    ]
    res = bass_utils.run_bass_kernel_spmd(
        nc, in_maps, core_ids=list(range(B)), trace=trace, **kwargs)
    outs = np.stack([np.asarray(r["out"]) for r in res.results], axis=0)
    return outs.astype(np.float32), res


def kernel(**inputs):
    outs, _ = _run(inputs, trace=False)
    return outs
